# revision 29
# baseline (speedup 1.0000x reference)
"""Contrastive loss (cosine similarity) Trainium2 Bass kernel.

Shapes (hardcoded): anchor [1024, 4096] f32, positive [1024, 8, 4096] f32,
negative [1024, 64, 4096] f32. Output: scalar f32 loss.

Strategy: pure data-parallel over the batch dim across 8 NeuronCores
(128 rows each). Per core, stream the 72 candidate vectors (8 pos + 64 neg)
as [128, 1, 4096] 2 MB tiles; for each candidate
  - DVE scalar_tensor_tensor: prod = v*a, dot = sum_free(prod)   (1 pass)
  - ACT activation(Square, accum_out): normsq = sum_free(v^2)    (1 pass)
Both engines overlap with the HBM DMA stream (~146 MB/core), which is the
roofline (358 GB/s/core -> ~428 us). The kernel ships the raw per-row
dots[72] and normsq[73] back to the host (75 KB/core) and the host does
the cheap cosine/log-softmax epilogue in float64 - this removes the
on-chip sqrt/exp/ln chain (+2 ACT table loads) from the critical tail.

Trace-driven tail tuning: 4 MB (2-candidate) transfers sustain 356.5
GB/s on the HWDGE rings; 2 MB transfers only reach 337 (16 KB
descriptors pay more per-packet overhead). But with uniform 4 MB chunks
the drain is gated on DVE buffer recycling (4.43 us/candidate STT),
stretching the tail ~12 us past the DMA stream end. Hybrid: bulk as
34 x 4 MB chunks (4-buffer ring), final 4 candidates as 8 x 1 MB half-D
chunks in their own 5-buffer ring with split accumulators (host sums
the halves), so tail arrivals stay line-rate-paced and the last DVE op
after the final arrival is ~2.3 us.

Junk elementwise outputs (prod/sq) are single shared bf16 tiles - WAW on
one engine is program order, costs nothing.
"""

import sys

if "/opt/trn_rl_repo" not in sys.path:
    sys.path.insert(0, "/opt/trn_rl_repo")

import numpy as np

import concourse.bass as bass
import concourse.mybir as mybir
import concourse.tile as tile
from concourse.bass_utils import run_bass_kernel_spmd

B, P, N, D = 1024, 8, 64, 4096
NCORES = 8
BS = B // NCORES  # 128 batch rows per core == SBUF partition count
J = P + N  # 72 candidates per row
TEMP = 0.1
CH = 2  # candidates per bulk DMA transfer (4 MB)
VBUFS = 4  # bulk buffer ring (4 x 4 MB)
HBUFS = 5  # tail piece buffer ring (5 x 1 MB slots)
# how many D-pieces each tail candidate streams as (rest are full-D).
# ({70:2, 71:4} quarter-tail measured statistically identical - 380.2us
# vs 379.0/379.8us for this config at equal ~422 GB/s draws.)
TAIL_SPLITS = {68: 2, 69: 2, 70: 2, 71: 2}
# Offloading dots to GPSIMD does NOT work: TENSOR_SCALAR_PTR is not a
# valid Pool-engine opcode on TRN2 (walrus codegen asserts).
USE_POOL_DOTS = False
NFULL = min(TAIL_SPLITS)  # candidates computed as one full-D op
NPIECES = NFULL + sum(TAIL_SPLITS.values())  # accum columns per quantity

# accumulator column layout in the [BS, ACC_W] output tile
DOT0 = 0  # dots: piece i at col i (fulls first, then tail pieces in order)
NSQ0 = 88  # normsq: same layout shifted by 88
ANSQ = NSQ0 + NPIECES  # anchor normsq column
ACC_W = 176

F32 = mybir.dt.float32
BF16 = mybir.dt.bfloat16
ALU = mybir.AluOpType
ACTF = mybir.ActivationFunctionType


def build_bass():
    nc = bass.Bass()
    anchor = nc.dram_tensor("anchor", (BS, D), F32, kind="ExternalInput")
    positive = nc.dram_tensor("positive", (BS, P, D), F32, kind="ExternalInput")
    negative = nc.dram_tensor("negative", (BS, N, D), F32, kind="ExternalInput")
    acc_out = nc.dram_tensor("acc", (BS, ACC_W), F32, kind="ExternalOutput")

    with tile.TileContext(nc) as tc:
        with (
            tc.tile_pool(name="vload", bufs=VBUFS) as vpool,
            tc.tile_pool(name="small", bufs=1) as small,
        ):
            a_tile = small.tile([BS, D], F32)
            nc.sync.dma_start(out=a_tile, in_=anchor[:, :])

            acc = small.tile([BS, ACC_W], F32)

            # single shared junk outputs; WAW per engine == program order.
            # Each engine gets its own junk tile - sharing across engines
            # would add cross-engine WAW semaphores. (PSUM dest for the ACT
            # junk would free SBUF but walrus SIGABRTs on PSUM-dest
            # ACTIVATE with accum_out in this build.)
            prod = small.tile([BS, D], BF16, tag="prod")
            sq = small.tile([BS, D], BF16, tag="sqd")
            pjunk = prod  # unused unless USE_POOL_DOTS

            nc.scalar.activation(
                out=sq, in_=a_tile, func=ACTF.Square, accum_out=acc[:, ANSQ : ANSQ + 1]
            )

            # chunk list: (tensor, row index, nvec, d0, d1, [(dot,nsq) cols])
            # bulk: CH candidates per 4 MB transfer; tail: sub-D pieces
            chunks = []
            for c in range(0, NFULL, CH):
                tens, i0 = (positive, c) if c < P else (negative, c - P)
                cols = [(DOT0 + c + k, NSQ0 + c + k) for k in range(CH)]
                chunks.append((tens, i0, CH, 0, D, cols))
            pcol = NFULL
            for c in range(NFULL, J):
                tens, i0 = (positive, c) if c < P else (negative, c - P)
                nsplit = TAIL_SPLITS[c]
                w = D // nsplit
                for k in range(nsplit):
                    chunks.append(
                        (tens, i0, 1, k * w, (k + 1) * w,
                         [(DOT0 + pcol, NSQ0 + pcol)])
                    )
                    pcol += 1
            assert pcol == NPIECES

            # dma_start for chunk ci is emitted LEAD chunks ahead of that
            # chunk's compute ops so each ring's dispatches keep a small
            # lead over the compute backlog on the same engine.
            LEAD = 2
            vtiles = [None] * len(chunks)

            def emit_compute(ci):
                _, _, nvec, d0, d1, cols = chunks[ci]
                w = d1 - d0
                v = vtiles[ci]
                for k in range(nvec):
                    dcol, ncol = cols[k]
                    # first candidate of every odd bulk chunk: dot on Pool
                    on_pool = USE_POOL_DOTS and nvec == CH and ci % 2 == 1 and k == 0
                    eng = nc.gpsimd if on_pool else nc.vector
                    eng.scalar_tensor_tensor(
                        out=(pjunk if on_pool else prod)[:, 0:w],
                        in0=v[:, k, :],
                        scalar=1.0,
                        in1=a_tile[:, d0:d1],
                        op0=ALU.bypass,
                        op1=ALU.mult,
                        accum_out=acc[:, dcol : dcol + 1],
                    )
                    nc.scalar.activation(
                        out=sq[:, 0:w],
                        in_=v[:, k, :],
                        func=ACTF.Square,
                        accum_out=acc[:, ncol : ncol + 1],
                    )

            for ci, (tens, i0, nvec, d0, d1, _) in enumerate(chunks):
                if nvec == CH:
                    v = vpool.tile([BS, CH, D], F32, tag="v", bufs=VBUFS)
                else:
                    v = vpool.tile([BS, 1, d1 - d0], F32, tag="vh", bufs=HBUFS)
                vtiles[ci] = v
                # all transfers ride the SP (Sync) HWDGE ring: SP has no
                # compute, so a dispatch blocked on buffer recycling never
                # head-of-line-blocks squares/dots the way an ACT-ring
                # dispatch does (one queue sustains >420 GB/s; splitting
                # across SP+GPSIMD queues measured worse: SWDGE overhead
                # plus packet-level round-robin breaks arrival ordering)
                nc.sync.dma_start(out=v, in_=tens[:, i0 : i0 + nvec, d0:d1])
                if ci >= LEAD:
                    emit_compute(ci - LEAD)
            for ci in range(len(chunks) - LEAD, len(chunks)):
                emit_compute(ci)

            # ship the raw accumulators; host does the cosine/log-softmax.
            # (Splitting this into an early ACT-ring normsq DMA + late dots
            # DMA measured neutral: with half-D tail pieces ACT and DVE
            # finish within ~0.2 us of each other, and the split turns one
            # 704 B/partition write into two sub-512 B RMW writes.)
            nc.sync.dma_start(out=acc_out[:, :], in_=acc)

    return nc


def _split_waits_json(bir_bytes):
    """Rewrite BIR so no instruction carries more than one sync wait.

    The walrus build in this environment has a single sync-wait slot per ISA
    instruction ("Too many sync wait commands" otherwise). Tile emits 2-4
    waits on some instructions; hoist all but the last onto pure-wait
    EventSemaphore carrier instructions on the same engine, which preserves
    semantics (sequential waits on one engine == AND of conditions).
    """
    import json as _json

    bir = _json.loads(bir_bytes)
    ctr = 0
    for fn in bir["functions"]:
        for blk in fn["blocks"]:
            out = []
            for inst in blk["instructions"]:
                si = inst.get("sync_info")
                waits = (si or {}).get("on_wait") or []
                if len(waits) > 1:
                    for w in waits[:-1]:
                        ctr += 1
                        out.append(
                            {
                                "name": f"ws-{ctr}",
                                "opcode": "EventSemaphore",
                                "engine": inst["engine"],
                                "ins": [],
                                "outs": [],
                                "sync_info": {"on_update": [], "on_wait": [w]},
                            }
                        )
                    si["on_wait"] = waits[-1:]
                out.append(inst)
            blk["instructions"] = out
    return _json.dumps(bir).encode()


_NC_CACHE = None


def _get_nc():
    global _NC_CACHE
    if _NC_CACHE is None:
        nc = build_bass()
        orig = nc.to_json_bytes
        nc.to_json_bytes = lambda: _split_waits_json(orig())
        _NC_CACHE = nc
    return _NC_CACHE


def _host_epilogue(accs):
    """accs: list of [BS, ACC_W] f32 per core -> scalar f32 loss."""
    acc = np.concatenate(accs, axis=0).astype(np.float64)  # [B, ACC_W]
    dots = np.empty((B, J))
    nsq = np.empty((B, J))
    dots[:, :NFULL] = acc[:, DOT0 : DOT0 + NFULL]
    nsq[:, :NFULL] = acc[:, NSQ0 : NSQ0 + NFULL]
    pcol = NFULL
    for c in range(NFULL, J):
        n = TAIL_SPLITS[c]
        dots[:, c] = acc[:, DOT0 + pcol : DOT0 + pcol + n].sum(axis=1)
        nsq[:, c] = acc[:, NSQ0 + pcol : NSQ0 + pcol + n].sum(axis=1)
        pcol += n
    a_nsq = acc[:, ANSQ]
    sims = dots / (TEMP * np.sqrt(nsq) * np.sqrt(a_nsq)[:, None])
    m = sims.max(axis=1)
    lse = m + np.log(np.exp(sims - m[:, None]).sum(axis=1))
    losses = lse - sims[:, :P].mean(axis=1)
    return np.asarray(losses.mean(), dtype=np.float32)


def run(anchor, positive, negative, trace=False, trace_cores=None):
    """Run on 8 cores; returns (loss ndarray, BassKernelResults)."""
    anchor = np.ascontiguousarray(anchor, dtype=np.float32)
    positive = np.ascontiguousarray(positive, dtype=np.float32)
    negative = np.ascontiguousarray(negative, dtype=np.float32)
    in_maps = []
    for c in range(NCORES):
        sl = slice(c * BS, (c + 1) * BS)
        in_maps.append(
            {
                "anchor": np.ascontiguousarray(anchor[sl]),
                "positive": np.ascontiguousarray(positive[sl]),
                "negative": np.ascontiguousarray(negative[sl]),
            }
        )
    res = run_bass_kernel_spmd(
        _get_nc(),
        in_maps,
        core_ids=list(range(NCORES)),
        trace=trace,
        trace_cores=trace_cores,
    )
    out = _host_epilogue([r["acc"] for r in res.results])
    return out, res


def kernel(anchor, positive, negative):
    out, _ = run(anchor, positive, negative)
    return out


# revision 30
# speedup vs baseline: 1.3894x; 1.3894x over previous
"""Contrastive loss (cosine similarity) Trainium2 Bass kernel.

Shapes (hardcoded): anchor [1024, 4096] f32, positive [1024, 8, 4096] f32,
negative [1024, 64, 4096] f32. Output: scalar f32 loss.

Strategy: pure data-parallel over the batch dim across 8 NeuronCores
(128 rows each). Per core, stream the 72 candidate vectors (8 pos + 64 neg)
as [128, 1, 4096] 2 MB tiles; for each candidate
  - DVE scalar_tensor_tensor: prod = v*a, dot = sum_free(prod)   (1 pass)
  - ACT activation(Square, accum_out): normsq = sum_free(v^2)    (1 pass)
Both engines overlap with the HBM DMA stream (~146 MB/core), which is
the roofline. With every transfer dispatched from the compute-free SP
(Sync) engine, one HWDGE queue sustains 421-423 GB/s = 98.5% of the
16-SDMA-engine aggregate ceiling (16 x 26.8 GB/s); quiet-chip exec is
~379 us. (The chip also has a contended regime, ~230-340 GB/s, set by
external tenants - same trace shape, just a slower gap-free stream.)
The kernel ships the raw per-row dots[72] and normsq[73] back to the
host (90 KB/core) and the host does the cheap cosine/log-softmax
epilogue in float64 - this removes the on-chip sqrt/exp/ln chain
(+2 ACT table loads) from the critical tail.

Trace-driven tail tuning: 4 MB (2-candidate) transfers are the
descriptor sweet spot; all-2MB measured ~20% slower. But with uniform
4 MB chunks the drain is gated on DVE buffer recycling (4.43 us per
candidate dot), stretching the tail ~12 us past the DMA stream end.
Hybrid: bulk as 34 x 4 MB chunks (4-buffer ring), final 4 candidates
as 8 x 1 MB half-D chunks in their own 5-buffer ring with split
accumulators (host sums the halves), so tail arrivals stay
line-rate-paced and the last DVE op after the final arrival is ~2.3 us.

Junk elementwise outputs (prod/sq) are single shared bf16 tiles - WAW on
one engine is program order, costs nothing.
"""

import sys

if "/opt/trn_rl_repo" not in sys.path:
    sys.path.insert(0, "/opt/trn_rl_repo")

import numpy as np

import concourse.bass as bass
import concourse.mybir as mybir
import concourse.tile as tile
from concourse.bass_utils import run_bass_kernel_spmd

B, P, N, D = 1024, 8, 64, 4096
NCORES = 8
BS = B // NCORES  # 128 batch rows per core == SBUF partition count
J = P + N  # 72 candidates per row
TEMP = 0.1
CH = 2  # candidates per bulk DMA transfer (4 MB)
VBUFS = 4  # bulk buffer ring (4 x 4 MB)
HBUFS = 5  # tail piece buffer ring (5 x 1 MB slots)
# how many D-pieces each tail candidate streams as (rest are full-D).
# ({70:2, 71:4} quarter-tail measured statistically identical - 380.2us
# vs 379.0/379.8us for this config at equal ~422 GB/s draws.)
TAIL_SPLITS = {68: 2, 69: 2, 70: 2, 71: 2}
# Offloading dots to GPSIMD does NOT work: TENSOR_SCALAR_PTR is not a
# valid Pool-engine opcode on TRN2 (walrus codegen asserts).
USE_POOL_DOTS = False
NFULL = min(TAIL_SPLITS)  # candidates computed as one full-D op
NPIECES = NFULL + sum(TAIL_SPLITS.values())  # accum columns per quantity

# accumulator column layout in the [BS, ACC_W] output tile
DOT0 = 0  # dots: piece i at col i (fulls first, then tail pieces in order)
NSQ0 = 88  # normsq: same layout shifted by 88
ANSQ = NSQ0 + NPIECES  # anchor normsq column
ACC_W = 176

F32 = mybir.dt.float32
BF16 = mybir.dt.bfloat16
ALU = mybir.AluOpType
ACTF = mybir.ActivationFunctionType


def build_bass():
    nc = bass.Bass()
    anchor = nc.dram_tensor("anchor", (BS, D), F32, kind="ExternalInput")
    positive = nc.dram_tensor("positive", (BS, P, D), F32, kind="ExternalInput")
    negative = nc.dram_tensor("negative", (BS, N, D), F32, kind="ExternalInput")
    acc_out = nc.dram_tensor("acc", (BS, ACC_W), F32, kind="ExternalOutput")

    with tile.TileContext(nc) as tc:
        with (
            tc.tile_pool(name="vload", bufs=VBUFS) as vpool,
            tc.tile_pool(name="small", bufs=1) as small,
        ):
            a_tile = small.tile([BS, D], F32)
            nc.sync.dma_start(out=a_tile, in_=anchor[:, :])

            acc = small.tile([BS, ACC_W], F32)

            # single shared junk outputs; WAW per engine == program order.
            # Each engine gets its own junk tile - sharing across engines
            # would add cross-engine WAW semaphores. (PSUM dest for the ACT
            # junk would free SBUF but walrus SIGABRTs on PSUM-dest
            # ACTIVATE with accum_out in this build.)
            prod = small.tile([BS, D], BF16, tag="prod")
            sq = small.tile([BS, D], BF16, tag="sqd")
            pjunk = prod  # unused unless USE_POOL_DOTS

            nc.scalar.activation(
                out=sq, in_=a_tile, func=ACTF.Square, accum_out=acc[:, ANSQ : ANSQ + 1]
            )

            # chunk list: (tensor, row index, nvec, d0, d1, [(dot,nsq) cols])
            # bulk: CH candidates per 4 MB transfer; tail: sub-D pieces
            chunks = []
            for c in range(0, NFULL, CH):
                tens, i0 = (positive, c) if c < P else (negative, c - P)
                cols = [(DOT0 + c + k, NSQ0 + c + k) for k in range(CH)]
                chunks.append((tens, i0, CH, 0, D, cols))
            pcol = NFULL
            for c in range(NFULL, J):
                tens, i0 = (positive, c) if c < P else (negative, c - P)
                nsplit = TAIL_SPLITS[c]
                w = D // nsplit
                for k in range(nsplit):
                    chunks.append(
                        (tens, i0, 1, k * w, (k + 1) * w,
                         [(DOT0 + pcol, NSQ0 + pcol)])
                    )
                    pcol += 1
            assert pcol == NPIECES

            # dma_start for chunk ci is emitted LEAD chunks ahead of that
            # chunk's compute ops so each ring's dispatches keep a small
            # lead over the compute backlog on the same engine.
            LEAD = 2
            vtiles = [None] * len(chunks)

            def emit_compute(ci):
                _, _, nvec, d0, d1, cols = chunks[ci]
                w = d1 - d0
                v = vtiles[ci]
                for k in range(nvec):
                    dcol, ncol = cols[k]
                    # first candidate of every odd bulk chunk: dot on Pool
                    on_pool = USE_POOL_DOTS and nvec == CH and ci % 2 == 1 and k == 0
                    eng = nc.gpsimd if on_pool else nc.vector
                    eng.scalar_tensor_tensor(
                        out=(pjunk if on_pool else prod)[:, 0:w],
                        in0=v[:, k, :],
                        scalar=1.0,
                        in1=a_tile[:, d0:d1],
                        op0=ALU.bypass,
                        op1=ALU.mult,
                        accum_out=acc[:, dcol : dcol + 1],
                    )
                    nc.scalar.activation(
                        out=sq[:, 0:w],
                        in_=v[:, k, :],
                        func=ACTF.Square,
                        accum_out=acc[:, ncol : ncol + 1],
                    )

            for ci, (tens, i0, nvec, d0, d1, _) in enumerate(chunks):
                if nvec == CH:
                    v = vpool.tile([BS, CH, D], F32, tag="v", bufs=VBUFS)
                else:
                    v = vpool.tile([BS, 1, d1 - d0], F32, tag="vh", bufs=HBUFS)
                vtiles[ci] = v
                # all transfers ride the SP (Sync) HWDGE ring: SP has no
                # compute, so a dispatch blocked on buffer recycling never
                # head-of-line-blocks squares/dots the way an ACT-ring
                # dispatch does (one queue sustains >420 GB/s; splitting
                # across SP+GPSIMD queues measured worse: SWDGE overhead
                # plus packet-level round-robin breaks arrival ordering)
                nc.sync.dma_start(out=v, in_=tens[:, i0 : i0 + nvec, d0:d1])
                if ci >= LEAD:
                    emit_compute(ci - LEAD)
            for ci in range(len(chunks) - LEAD, len(chunks)):
                emit_compute(ci)

            # ship the raw accumulators; host does the cosine/log-softmax.
            # (Splitting this into an early ACT-ring normsq DMA + late dots
            # DMA measured neutral: with half-D tail pieces ACT and DVE
            # finish within ~0.2 us of each other, and the split turns one
            # 704 B/partition write into two sub-512 B RMW writes.)
            nc.sync.dma_start(out=acc_out[:, :], in_=acc)

    return nc


def _split_waits_json(bir_bytes):
    """Rewrite BIR so no instruction carries more than one sync wait.

    The walrus build in this environment has a single sync-wait slot per ISA
    instruction ("Too many sync wait commands" otherwise). Tile emits 2-4
    waits on some instructions; hoist all but the last onto pure-wait
    EventSemaphore carrier instructions on the same engine, which preserves
    semantics (sequential waits on one engine == AND of conditions).
    """
    import json as _json

    bir = _json.loads(bir_bytes)
    ctr = 0
    for fn in bir["functions"]:
        for blk in fn["blocks"]:
            out = []
            for inst in blk["instructions"]:
                si = inst.get("sync_info")
                waits = (si or {}).get("on_wait") or []
                if len(waits) > 1:
                    for w in waits[:-1]:
                        ctr += 1
                        out.append(
                            {
                                "name": f"ws-{ctr}",
                                "opcode": "EventSemaphore",
                                "engine": inst["engine"],
                                "ins": [],
                                "outs": [],
                                "sync_info": {"on_update": [], "on_wait": [w]},
                            }
                        )
                    si["on_wait"] = waits[-1:]
                out.append(inst)
            blk["instructions"] = out
    return _json.dumps(bir).encode()


_NC_CACHE = None


def _get_nc():
    global _NC_CACHE
    if _NC_CACHE is None:
        nc = build_bass()
        orig = nc.to_json_bytes
        nc.to_json_bytes = lambda: _split_waits_json(orig())
        _NC_CACHE = nc
    return _NC_CACHE


def _host_epilogue(accs):
    """accs: list of [BS, ACC_W] f32 per core -> scalar f32 loss."""
    acc = np.concatenate(accs, axis=0).astype(np.float64)  # [B, ACC_W]
    dots = np.empty((B, J))
    nsq = np.empty((B, J))
    dots[:, :NFULL] = acc[:, DOT0 : DOT0 + NFULL]
    nsq[:, :NFULL] = acc[:, NSQ0 : NSQ0 + NFULL]
    pcol = NFULL
    for c in range(NFULL, J):
        n = TAIL_SPLITS[c]
        dots[:, c] = acc[:, DOT0 + pcol : DOT0 + pcol + n].sum(axis=1)
        nsq[:, c] = acc[:, NSQ0 + pcol : NSQ0 + pcol + n].sum(axis=1)
        pcol += n
    a_nsq = acc[:, ANSQ]
    sims = dots / (TEMP * np.sqrt(nsq) * np.sqrt(a_nsq)[:, None])
    m = sims.max(axis=1)
    lse = m + np.log(np.exp(sims - m[:, None]).sum(axis=1))
    losses = lse - sims[:, :P].mean(axis=1)
    return np.asarray(losses.mean(), dtype=np.float32)


def run(anchor, positive, negative, trace=False, trace_cores=None):
    """Run on 8 cores; returns (loss ndarray, BassKernelResults)."""
    anchor = np.ascontiguousarray(anchor, dtype=np.float32)
    positive = np.ascontiguousarray(positive, dtype=np.float32)
    negative = np.ascontiguousarray(negative, dtype=np.float32)
    in_maps = []
    for c in range(NCORES):
        sl = slice(c * BS, (c + 1) * BS)
        in_maps.append(
            {
                "anchor": np.ascontiguousarray(anchor[sl]),
                "positive": np.ascontiguousarray(positive[sl]),
                "negative": np.ascontiguousarray(negative[sl]),
            }
        )
    res = run_bass_kernel_spmd(
        _get_nc(),
        in_maps,
        core_ids=list(range(NCORES)),
        trace=trace,
        trace_cores=trace_cores,
    )
    out = _host_epilogue([r["acc"] for r in res.results])
    return out, res


def kernel(anchor, positive, negative):
    out, _ = run(anchor, positive, negative)
    return out


# revision 31
# speedup vs baseline: 1.4102x; 1.0150x over previous
"""Contrastive loss (cosine similarity) Trainium2 Bass kernel.

Shapes (hardcoded): anchor [1024, 4096] f32, positive [1024, 8, 4096] f32,
negative [1024, 64, 4096] f32. Output: scalar f32 loss.

Strategy: pure data-parallel over the batch dim across 8 NeuronCores
(128 rows each). Per core, stream the 72 candidate vectors (8 pos + 64 neg)
as [128, 1, 4096] 2 MB tiles; for each candidate
  - DVE scalar_tensor_tensor: prod = v*a, dot = sum_free(prod)   (1 pass)
  - ACT activation(Square, accum_out): normsq = sum_free(v^2)    (1 pass)
Both engines overlap with the HBM DMA stream (~146 MB/core), which is
the roofline. With every transfer dispatched from the compute-free SP
(Sync) engine, one HWDGE queue sustains 421-423 GB/s = 98.5% of the
16-SDMA-engine aggregate ceiling (16 x 26.8 GB/s); quiet-chip exec is
~379 us. (The chip also has a contended regime, ~230-340 GB/s, set by
external tenants - same trace shape, just a slower gap-free stream.)
The kernel ships the raw per-row dots[72] and normsq[73] back to the
host (90 KB/core) and the host does the cheap cosine/log-softmax
epilogue in float64 - this removes the on-chip sqrt/exp/ln chain
(+2 ACT table loads) from the critical tail.

Trace-driven tail tuning: 4 MB (2-candidate) transfers are the
descriptor sweet spot; all-2MB measured ~20% slower. But with uniform
4 MB chunks the drain is gated on DVE buffer recycling (4.43 us per
candidate dot), stretching the tail ~12 us past the DMA stream end.
Hybrid: bulk as 34 x 4 MB chunks (4-buffer ring), final 4 candidates
as 8 x 1 MB half-D chunks in their own 5-buffer ring with split
accumulators (host sums the halves), so tail arrivals stay
line-rate-paced and the last DVE op after the final arrival is ~2.3 us.

Junk elementwise outputs (prod/sq) are single shared bf16 tiles - WAW on
one engine is program order, costs nothing.
"""

import sys

if "/opt/trn_rl_repo" not in sys.path:
    sys.path.insert(0, "/opt/trn_rl_repo")

import numpy as np

import concourse.bass as bass
import concourse.mybir as mybir
import concourse.tile as tile
from concourse.bass_utils import run_bass_kernel_spmd

B, P, N, D = 1024, 8, 64, 4096
NCORES = 8
BS = B // NCORES  # 128 batch rows per core == SBUF partition count
J = P + N  # 72 candidates per row
TEMP = 0.1
CH = 2  # candidates per bulk DMA transfer (4 MB)
VBUFS = 3  # bulk buffer ring (3 x 4 MB; ~9.9 us recycle slack mid-stream)
HBUFS = 8  # tail piece buffers — one FRESH slot per piece: piece dispatches
# never wait on compute, killing the trace-confirmed 1-5 us tail leak
# (pieces 6-8 used to gate on pieces 1-3's dots via DVE's in-order queue)
# how many D-pieces each tail candidate streams as (rest are full-D).
# ({70:2, 71:4} quarter-tail measured statistically identical - 380.2us
# vs 379.0/379.8us for this config at equal ~422 GB/s draws.)
TAIL_SPLITS = {68: 2, 69: 2, 70: 2, 71: 2}
# Offloading dots to GPSIMD does NOT work: TENSOR_SCALAR_PTR is not a
# valid Pool-engine opcode on TRN2 (walrus codegen asserts).
USE_POOL_DOTS = False
NFULL = min(TAIL_SPLITS)  # candidates computed as one full-D op
NPIECES = NFULL + sum(TAIL_SPLITS.values())  # accum columns per quantity

# accumulator column layout in the [BS, ACC_W] output tile
DOT0 = 0  # dots: piece i at col i (fulls first, then tail pieces in order)
NSQ0 = 88  # normsq: same layout shifted by 88
ANSQ = NSQ0 + NPIECES  # anchor normsq column
ACC_W = 176

F32 = mybir.dt.float32
BF16 = mybir.dt.bfloat16
ALU = mybir.AluOpType
ACTF = mybir.ActivationFunctionType


def build_bass():
    nc = bass.Bass()
    anchor = nc.dram_tensor("anchor", (BS, D), F32, kind="ExternalInput")
    positive = nc.dram_tensor("positive", (BS, P, D), F32, kind="ExternalInput")
    negative = nc.dram_tensor("negative", (BS, N, D), F32, kind="ExternalInput")
    acc_out = nc.dram_tensor("acc", (BS, ACC_W), F32, kind="ExternalOutput")

    with tile.TileContext(nc) as tc:
        with (
            tc.tile_pool(name="vload", bufs=VBUFS) as vpool,
            tc.tile_pool(name="small", bufs=1) as small,
        ):
            a_tile = small.tile([BS, D], F32)
            nc.sync.dma_start(out=a_tile, in_=anchor[:, :])

            acc = small.tile([BS, ACC_W], F32)

            # single shared junk outputs; WAW per engine == program order.
            # Each engine gets its own junk tile - sharing across engines
            # would add cross-engine WAW semaphores. (PSUM dest for the ACT
            # junk would free SBUF but walrus SIGABRTs on PSUM-dest
            # ACTIVATE with accum_out in this build.)
            prod = small.tile([BS, D], BF16, tag="prod")
            sq = small.tile([BS, D], BF16, tag="sqd")
            pjunk = prod  # unused unless USE_POOL_DOTS

            nc.scalar.activation(
                out=sq, in_=a_tile, func=ACTF.Square, accum_out=acc[:, ANSQ : ANSQ + 1]
            )

            # chunk list: (tensor, row index, nvec, d0, d1, [(dot,nsq) cols])
            # bulk: CH candidates per 4 MB transfer; tail: sub-D pieces
            chunks = []
            for c in range(0, NFULL, CH):
                tens, i0 = (positive, c) if c < P else (negative, c - P)
                cols = [(DOT0 + c + k, NSQ0 + c + k) for k in range(CH)]
                chunks.append((tens, i0, CH, 0, D, cols))
            pcol = NFULL
            for c in range(NFULL, J):
                tens, i0 = (positive, c) if c < P else (negative, c - P)
                nsplit = TAIL_SPLITS[c]
                w = D // nsplit
                for k in range(nsplit):
                    chunks.append(
                        (tens, i0, 1, k * w, (k + 1) * w,
                         [(DOT0 + pcol, NSQ0 + pcol)])
                    )
                    pcol += 1
            assert pcol == NPIECES

            # dma_start for chunk ci is emitted LEAD chunks ahead of that
            # chunk's compute ops so each ring's dispatches keep a small
            # lead over the compute backlog on the same engine.
            LEAD = 2
            vtiles = [None] * len(chunks)

            def emit_compute(ci):
                _, _, nvec, d0, d1, cols = chunks[ci]
                w = d1 - d0
                v = vtiles[ci]
                for k in range(nvec):
                    dcol, ncol = cols[k]
                    # first candidate of every odd bulk chunk: dot on Pool
                    on_pool = USE_POOL_DOTS and nvec == CH and ci % 2 == 1 and k == 0
                    eng = nc.gpsimd if on_pool else nc.vector
                    eng.scalar_tensor_tensor(
                        out=(pjunk if on_pool else prod)[:, 0:w],
                        in0=v[:, k, :],
                        scalar=1.0,
                        in1=a_tile[:, d0:d1],
                        op0=ALU.bypass,
                        op1=ALU.mult,
                        accum_out=acc[:, dcol : dcol + 1],
                    )
                    nc.scalar.activation(
                        out=sq[:, 0:w],
                        in_=v[:, k, :],
                        func=ACTF.Square,
                        accum_out=acc[:, ncol : ncol + 1],
                    )

            for ci, (tens, i0, nvec, d0, d1, _) in enumerate(chunks):
                if nvec == CH:
                    v = vpool.tile([BS, CH, D], F32, tag="v", bufs=VBUFS)
                else:
                    v = vpool.tile([BS, 1, d1 - d0], F32, tag="vh", bufs=HBUFS)
                vtiles[ci] = v
                # all transfers ride the SP (Sync) HWDGE ring: SP has no
                # compute, so a dispatch blocked on buffer recycling never
                # head-of-line-blocks squares/dots the way an ACT-ring
                # dispatch does (one queue sustains >420 GB/s; splitting
                # across SP+GPSIMD queues measured worse: SWDGE overhead
                # plus packet-level round-robin breaks arrival ordering)
                nc.sync.dma_start(out=v, in_=tens[:, i0 : i0 + nvec, d0:d1])
                if ci >= LEAD:
                    emit_compute(ci - LEAD)
            for ci in range(len(chunks) - LEAD, len(chunks)):
                emit_compute(ci)

            # ship the raw accumulators; host does the cosine/log-softmax.
            # (Splitting this into an early ACT-ring normsq DMA + late dots
            # DMA measured neutral: with half-D tail pieces ACT and DVE
            # finish within ~0.2 us of each other, and the split turns one
            # 704 B/partition write into two sub-512 B RMW writes.)
            nc.sync.dma_start(out=acc_out[:, :], in_=acc)

    return nc


def _split_waits_json(bir_bytes):
    """Rewrite BIR so no instruction carries more than one sync wait.

    The walrus build in this environment has a single sync-wait slot per ISA
    instruction ("Too many sync wait commands" otherwise). Tile emits 2-4
    waits on some instructions; hoist all but the last onto pure-wait
    EventSemaphore carrier instructions on the same engine, which preserves
    semantics (sequential waits on one engine == AND of conditions).
    """
    import json as _json

    bir = _json.loads(bir_bytes)
    ctr = 0
    for fn in bir["functions"]:
        for blk in fn["blocks"]:
            out = []
            for inst in blk["instructions"]:
                si = inst.get("sync_info")
                waits = (si or {}).get("on_wait") or []
                if len(waits) > 1:
                    for w in waits[:-1]:
                        ctr += 1
                        out.append(
                            {
                                "name": f"ws-{ctr}",
                                "opcode": "EventSemaphore",
                                "engine": inst["engine"],
                                "ins": [],
                                "outs": [],
                                "sync_info": {"on_update": [], "on_wait": [w]},
                            }
                        )
                    si["on_wait"] = waits[-1:]
                out.append(inst)
            blk["instructions"] = out
    return _json.dumps(bir).encode()


_NC_CACHE = None


def _get_nc():
    global _NC_CACHE
    if _NC_CACHE is None:
        nc = build_bass()
        orig = nc.to_json_bytes
        nc.to_json_bytes = lambda: _split_waits_json(orig())
        _NC_CACHE = nc
    return _NC_CACHE


def _host_epilogue(accs):
    """accs: list of [BS, ACC_W] f32 per core -> scalar f32 loss."""
    acc = np.concatenate(accs, axis=0).astype(np.float64)  # [B, ACC_W]
    dots = np.empty((B, J))
    nsq = np.empty((B, J))
    dots[:, :NFULL] = acc[:, DOT0 : DOT0 + NFULL]
    nsq[:, :NFULL] = acc[:, NSQ0 : NSQ0 + NFULL]
    pcol = NFULL
    for c in range(NFULL, J):
        n = TAIL_SPLITS[c]
        dots[:, c] = acc[:, DOT0 + pcol : DOT0 + pcol + n].sum(axis=1)
        nsq[:, c] = acc[:, NSQ0 + pcol : NSQ0 + pcol + n].sum(axis=1)
        pcol += n
    a_nsq = acc[:, ANSQ]
    sims = dots / (TEMP * np.sqrt(nsq) * np.sqrt(a_nsq)[:, None])
    m = sims.max(axis=1)
    lse = m + np.log(np.exp(sims - m[:, None]).sum(axis=1))
    losses = lse - sims[:, :P].mean(axis=1)
    return np.asarray(losses.mean(), dtype=np.float32)


def run(anchor, positive, negative, trace=False, trace_cores=None):
    """Run on 8 cores; returns (loss ndarray, BassKernelResults)."""
    anchor = np.ascontiguousarray(anchor, dtype=np.float32)
    positive = np.ascontiguousarray(positive, dtype=np.float32)
    negative = np.ascontiguousarray(negative, dtype=np.float32)
    in_maps = []
    for c in range(NCORES):
        sl = slice(c * BS, (c + 1) * BS)
        in_maps.append(
            {
                "anchor": np.ascontiguousarray(anchor[sl]),
                "positive": np.ascontiguousarray(positive[sl]),
                "negative": np.ascontiguousarray(negative[sl]),
            }
        )
    res = run_bass_kernel_spmd(
        _get_nc(),
        in_maps,
        core_ids=list(range(NCORES)),
        trace=trace,
        trace_cores=trace_cores,
    )
    out = _host_epilogue([r["acc"] for r in res.results])
    return out, res


def kernel(anchor, positive, negative):
    out, _ = run(anchor, positive, negative)
    return out


# revision 32
# speedup vs baseline: 1.4366x; 1.0187x over previous
"""Contrastive loss (cosine similarity) Trainium2 Bass kernel.

Shapes (hardcoded): anchor [1024, 4096] f32, positive [1024, 8, 4096] f32,
negative [1024, 64, 4096] f32. Output: scalar f32 loss.

Strategy: pure data-parallel over the batch dim across 8 NeuronCores
(128 rows each). Per core, stream the 72 candidate vectors (8 pos + 64 neg)
as [128, 1, 4096] 2 MB tiles; for each candidate
  - DVE scalar_tensor_tensor: prod = v*a, dot = sum_free(prod)   (1 pass)
  - ACT activation(Square, accum_out): normsq = sum_free(v^2)    (1 pass)
Both engines overlap with the HBM DMA stream (~146 MB/core), which is
the roofline. With every transfer dispatched from the compute-free SP
(Sync) engine, one HWDGE queue sustains 421-423 GB/s = 98.5% of the
16-SDMA-engine aggregate ceiling (16 x 26.8 GB/s); quiet-chip exec is
~379 us. (The chip also has a contended regime, ~230-340 GB/s, set by
external tenants - same trace shape, just a slower gap-free stream.)
The kernel ships the raw per-row dots[72] and normsq[73] back to the
host (90 KB/core) and the host does the cheap cosine/log-softmax
epilogue in float64 - this removes the on-chip sqrt/exp/ln chain
(+2 ACT table loads) from the critical tail.

Trace-driven tail tuning: 4 MB (2-candidate) transfers are the
descriptor sweet spot; all-2MB measured ~20% slower. But with uniform
4 MB chunks the drain is gated on DVE buffer recycling (4.43 us per
candidate dot), stretching the tail ~12 us past the DMA stream end.
Hybrid: bulk as 34 x 4 MB chunks (4-buffer ring), final 4 candidates
as 8 x 1 MB half-D chunks in their own 5-buffer ring with split
accumulators (host sums the halves), so tail arrivals stay
line-rate-paced and the last DVE op after the final arrival is ~2.3 us.

Junk elementwise outputs (prod/sq) are single shared bf16 tiles - WAW on
one engine is program order, costs nothing.
"""

import sys

if "/opt/trn_rl_repo" not in sys.path:
    sys.path.insert(0, "/opt/trn_rl_repo")

import numpy as np

import concourse.bass as bass
import concourse.mybir as mybir
import concourse.tile as tile
from concourse.bass_utils import run_bass_kernel_spmd

B, P, N, D = 1024, 8, 64, 4096
NCORES = 8
BS = B // NCORES  # 128 batch rows per core == SBUF partition count
J = P + N  # 72 candidates per row
TEMP = 0.1
CH = 2  # candidates per bulk DMA transfer (4 MB)
VBUFS = 4  # bulk buffer ring (4 x 4 MB)
HBUFS = 5  # tail piece buffer ring (5 x 1 MB slots)
# (VBUFS=3 + 8 fresh tail slots removes the piece-dispatch compute-gating
# but measured rate-normalized-equal, exposes DMA-sem-recycle gating
# underneath, and halves the fast-regime jitter margin - not worth it.)
# how many D-pieces each tail candidate streams as (rest are full-D).
# ({70:2, 71:4} quarter-tail measured statistically identical - 380.2us
# vs 379.0/379.8us for this config at equal ~422 GB/s draws.)
TAIL_SPLITS = {68: 2, 69: 2, 70: 2, 71: 2}
# Offloading dots to GPSIMD does NOT work: TENSOR_SCALAR_PTR is not a
# valid Pool-engine opcode on TRN2 (walrus codegen asserts).
USE_POOL_DOTS = False
NFULL = min(TAIL_SPLITS)  # candidates computed as one full-D op
NPIECES = NFULL + sum(TAIL_SPLITS.values())  # accum columns per quantity

# accumulator column layout in the [BS, ACC_W] output tile
DOT0 = 0  # dots: piece i at col i (fulls first, then tail pieces in order)
NSQ0 = 88  # normsq: same layout shifted by 88
ANSQ = NSQ0 + NPIECES  # anchor normsq column
ACC_W = 176

F32 = mybir.dt.float32
BF16 = mybir.dt.bfloat16
ALU = mybir.AluOpType
ACTF = mybir.ActivationFunctionType


def build_bass():
    nc = bass.Bass()
    anchor = nc.dram_tensor("anchor", (BS, D), F32, kind="ExternalInput")
    positive = nc.dram_tensor("positive", (BS, P, D), F32, kind="ExternalInput")
    negative = nc.dram_tensor("negative", (BS, N, D), F32, kind="ExternalInput")
    acc_out = nc.dram_tensor("acc", (BS, ACC_W), F32, kind="ExternalOutput")

    with tile.TileContext(nc) as tc:
        with (
            tc.tile_pool(name="vload", bufs=VBUFS) as vpool,
            tc.tile_pool(name="small", bufs=1) as small,
        ):
            a_tile = small.tile([BS, D], F32)
            nc.sync.dma_start(out=a_tile, in_=anchor[:, :])

            acc = small.tile([BS, ACC_W], F32)

            # single shared junk outputs; WAW per engine == program order.
            # Each engine gets its own junk tile - sharing across engines
            # would add cross-engine WAW semaphores. (PSUM dest for the ACT
            # junk would free SBUF but walrus SIGABRTs on PSUM-dest
            # ACTIVATE with accum_out in this build.)
            prod = small.tile([BS, D], BF16, tag="prod")
            sq = small.tile([BS, D], BF16, tag="sqd")
            pjunk = prod  # unused unless USE_POOL_DOTS

            nc.scalar.activation(
                out=sq, in_=a_tile, func=ACTF.Square, accum_out=acc[:, ANSQ : ANSQ + 1]
            )

            # chunk list: (tensor, row index, nvec, d0, d1, [(dot,nsq) cols])
            # bulk: CH candidates per 4 MB transfer; tail: sub-D pieces
            chunks = []
            for c in range(0, NFULL, CH):
                tens, i0 = (positive, c) if c < P else (negative, c - P)
                cols = [(DOT0 + c + k, NSQ0 + c + k) for k in range(CH)]
                chunks.append((tens, i0, CH, 0, D, cols))
            pcol = NFULL
            for c in range(NFULL, J):
                tens, i0 = (positive, c) if c < P else (negative, c - P)
                nsplit = TAIL_SPLITS[c]
                w = D // nsplit
                for k in range(nsplit):
                    chunks.append(
                        (tens, i0, 1, k * w, (k + 1) * w,
                         [(DOT0 + pcol, NSQ0 + pcol)])
                    )
                    pcol += 1
            assert pcol == NPIECES

            # dma_start for chunk ci is emitted LEAD chunks ahead of that
            # chunk's compute ops so each ring's dispatches keep a small
            # lead over the compute backlog on the same engine.
            LEAD = 2
            vtiles = [None] * len(chunks)

            def emit_compute(ci):
                _, _, nvec, d0, d1, cols = chunks[ci]
                w = d1 - d0
                v = vtiles[ci]
                for k in range(nvec):
                    dcol, ncol = cols[k]
                    # first candidate of every odd bulk chunk: dot on Pool
                    on_pool = USE_POOL_DOTS and nvec == CH and ci % 2 == 1 and k == 0
                    eng = nc.gpsimd if on_pool else nc.vector
                    eng.scalar_tensor_tensor(
                        out=(pjunk if on_pool else prod)[:, 0:w],
                        in0=v[:, k, :],
                        scalar=1.0,
                        in1=a_tile[:, d0:d1],
                        op0=ALU.bypass,
                        op1=ALU.mult,
                        accum_out=acc[:, dcol : dcol + 1],
                    )
                    nc.scalar.activation(
                        out=sq[:, 0:w],
                        in_=v[:, k, :],
                        func=ACTF.Square,
                        accum_out=acc[:, ncol : ncol + 1],
                    )

            for ci, (tens, i0, nvec, d0, d1, _) in enumerate(chunks):
                if nvec == CH:
                    v = vpool.tile([BS, CH, D], F32, tag="v", bufs=VBUFS)
                else:
                    v = vpool.tile([BS, 1, d1 - d0], F32, tag="vh", bufs=HBUFS)
                vtiles[ci] = v
                # all transfers ride the SP (Sync) HWDGE ring: SP has no
                # compute, so a dispatch blocked on buffer recycling never
                # head-of-line-blocks squares/dots the way an ACT-ring
                # dispatch does (one queue sustains >420 GB/s; splitting
                # across SP+GPSIMD queues measured worse: SWDGE overhead
                # plus packet-level round-robin breaks arrival ordering)
                nc.sync.dma_start(out=v, in_=tens[:, i0 : i0 + nvec, d0:d1])
                if ci >= LEAD:
                    emit_compute(ci - LEAD)
            for ci in range(len(chunks) - LEAD, len(chunks)):
                emit_compute(ci)

            # ship the raw accumulators; host does the cosine/log-softmax.
            # (Splitting this into an early ACT-ring normsq DMA + late dots
            # DMA measured neutral: with half-D tail pieces ACT and DVE
            # finish within ~0.2 us of each other, and the split turns one
            # 704 B/partition write into two sub-512 B RMW writes.)
            nc.sync.dma_start(out=acc_out[:, :], in_=acc)

    return nc


def _split_waits_json(bir_bytes):
    """Rewrite BIR so no instruction carries more than one sync wait.

    The walrus build in this environment has a single sync-wait slot per ISA
    instruction ("Too many sync wait commands" otherwise). Tile emits 2-4
    waits on some instructions; hoist all but the last onto pure-wait
    EventSemaphore carrier instructions on the same engine, which preserves
    semantics (sequential waits on one engine == AND of conditions).
    """
    import json as _json

    bir = _json.loads(bir_bytes)
    ctr = 0
    for fn in bir["functions"]:
        for blk in fn["blocks"]:
            out = []
            for inst in blk["instructions"]:
                si = inst.get("sync_info")
                waits = (si or {}).get("on_wait") or []
                if len(waits) > 1:
                    for w in waits[:-1]:
                        ctr += 1
                        out.append(
                            {
                                "name": f"ws-{ctr}",
                                "opcode": "EventSemaphore",
                                "engine": inst["engine"],
                                "ins": [],
                                "outs": [],
                                "sync_info": {"on_update": [], "on_wait": [w]},
                            }
                        )
                    si["on_wait"] = waits[-1:]
                out.append(inst)
            blk["instructions"] = out
    return _json.dumps(bir).encode()


_NC_CACHE = None


def _get_nc():
    global _NC_CACHE
    if _NC_CACHE is None:
        nc = build_bass()
        orig = nc.to_json_bytes
        nc.to_json_bytes = lambda: _split_waits_json(orig())
        _NC_CACHE = nc
    return _NC_CACHE


def _host_epilogue(accs):
    """accs: list of [BS, ACC_W] f32 per core -> scalar f32 loss."""
    acc = np.concatenate(accs, axis=0).astype(np.float64)  # [B, ACC_W]
    dots = np.empty((B, J))
    nsq = np.empty((B, J))
    dots[:, :NFULL] = acc[:, DOT0 : DOT0 + NFULL]
    nsq[:, :NFULL] = acc[:, NSQ0 : NSQ0 + NFULL]
    pcol = NFULL
    for c in range(NFULL, J):
        n = TAIL_SPLITS[c]
        dots[:, c] = acc[:, DOT0 + pcol : DOT0 + pcol + n].sum(axis=1)
        nsq[:, c] = acc[:, NSQ0 + pcol : NSQ0 + pcol + n].sum(axis=1)
        pcol += n
    a_nsq = acc[:, ANSQ]
    sims = dots / (TEMP * np.sqrt(nsq) * np.sqrt(a_nsq)[:, None])
    m = sims.max(axis=1)
    lse = m + np.log(np.exp(sims - m[:, None]).sum(axis=1))
    losses = lse - sims[:, :P].mean(axis=1)
    return np.asarray(losses.mean(), dtype=np.float32)


def run(anchor, positive, negative, trace=False, trace_cores=None):
    """Run on 8 cores; returns (loss ndarray, BassKernelResults)."""
    anchor = np.ascontiguousarray(anchor, dtype=np.float32)
    positive = np.ascontiguousarray(positive, dtype=np.float32)
    negative = np.ascontiguousarray(negative, dtype=np.float32)
    in_maps = []
    for c in range(NCORES):
        sl = slice(c * BS, (c + 1) * BS)
        in_maps.append(
            {
                "anchor": np.ascontiguousarray(anchor[sl]),
                "positive": np.ascontiguousarray(positive[sl]),
                "negative": np.ascontiguousarray(negative[sl]),
            }
        )
    res = run_bass_kernel_spmd(
        _get_nc(),
        in_maps,
        core_ids=list(range(NCORES)),
        trace=trace,
        trace_cores=trace_cores,
    )
    out = _host_epilogue([r["acc"] for r in res.results])
    return out, res


def kernel(anchor, positive, negative):
    out, _ = run(anchor, positive, negative)
    return out
